# revision 26
# baseline (speedup 1.0000x reference)
"""Trainium2 Bass kernel for the DiscontinuityGNN (3-layer dense GAT + edge MLP).

Strategy: 1D row-parallel over the N=2048 nodes across 8 cores (256 rows each).
Activations are kept in transposed [features, nodes] layout so every matmul
consumes them directly as lhsT/rhs without on-device transposes.  Attention is
computed transposed ([q, p_local]): the per-q score s1 arrives as a per-
partition ACT bias, the per-p score s2 is broadcast across partitions with a
K=1 ones-matmul, masking happens after exp (identical to the reference's
-9e15 masking; max-subtraction is skipped since |e| <= ~5), and softmax
denominators come from a fused ones-row in the PV matmul.  Node features are
all-gathered on device between layers.  The edge classifier packs 4 q-blocks
x 3 channels into K=12 block-diagonal matmuls per row, packs 32 rows of
sigmoid inputs into one PSUM bank, and writes output with one DMA per batch.
"""

import os

import numpy as np

import concourse.mybir as mybir
import concourse.tile as tile
from concourse import bacc, bass_utils

F32 = mybir.dt.float32
AF = mybir.ActivationFunctionType
ALU = mybir.AluOpType

N = 2048
NC = 8            # cores
NL = N // NC      # local rows per core = 256
H = 4             # heads
HID = 64
OUT = 3
ALPHA = 0.2
FIN = [HID, H * HID, H * HID]
FO = [HID, HID, OUT]

_CACHE = {}


def _build():
    nc = bacc.Bacc("TRN2", target_bir_lowering=False, debug=False, num_devices=NC)

    inp = {}

    def din(name, shape):
        inp[name] = nc.dram_tensor(name, list(shape), F32, kind="ExternalInput")
        return inp[name]

    nfT_loc = din("nfT_loc", (10, NL))
    adjT_loc = din("adjT_loc", (N, NL))
    enc_W1 = din("enc_W1", (10, HID))
    enc_b1c = din("enc_b1c", (HID, 1))
    enc_W2 = din("enc_W2", (HID, HID))
    enc_b2c = din("enc_b2c", (HID, 1))
    gat_W, gat_a1r, gat_a2 = {}, {}, {}
    for layer in range(3):
        for h in range(H):
            gat_W[layer, h] = din(f"gat_W_{layer}_{h}", (FIN[layer], FO[layer]))
            gat_a1r[layer, h] = din(f"gat_a1r_{layer}_{h}", (128, FO[layer]))
            gat_a2[layer, h] = din(f"gat_a2_{layer}_{h}", (FO[layer], 1))
    W1blk = din("W1blk", (128, 128))
    W2blk = din("W2blk", (128, 32))
    b1c = din("b1c", (128, 1))
    b2c = din("b2c", (128, 1))
    ones128 = din("ones128", (1, 128))

    embT_out = nc.dram_tensor("embT_loc", [OUT, NL], F32, kind="ExternalOutput")
    ep_out = nc.dram_tensor("ep_loc", [NL, N], F32, kind="ExternalOutput")
    kdebug = os.environ.get("KDEBUG", "0") == "1"
    if kdebug:
        dbg_emb_i = nc.dram_tensor("dbg_emb_i", [128, 512], F32,
                                   kind="ExternalOutput")
        dbg_embP = nc.dram_tensor("dbg_embP", [128, 64], F32,
                                  kind="ExternalOutput")
        dbg_adf = nc.dram_tensor("dbg_adf", [128, 512], F32,
                                 kind="ExternalOutput")
        dbg_hid = nc.dram_tensor("dbg_hid", [128, 512], F32,
                                 kind="ExternalOutput")
        dbg_probs = nc.dram_tensor("dbg_probs", [128, 512], F32,
                                   kind="ExternalOutput")

    RG = [list(range(NC))]

    with tile.TileContext(nc) as tc:
        with (
            tc.tile_pool(name="const", bufs=1) as cpool,
            tc.tile_pool(name="big", bufs=1) as big,
            tc.tile_pool(name="work", bufs=2) as work,
            tc.tile_pool(name="att", bufs=3) as attp,
            tc.tile_pool(name="cls", bufs=3) as clsp,
            tc.tile_pool(name="ps_mm", bufs=2, space="PSUM") as ps_mm,
            tc.tile_pool(name="ps_s2", bufs=2, space="PSUM") as ps_s2,
            tc.tile_pool(name="ps_pv", bufs=2, space="PSUM") as ps_pv,
            tc.tile_pool(name="ps_pr", bufs=2, space="PSUM") as ps_pr,
            tc.tile_pool(name="dram", bufs=1, space="DRAM") as dram,
        ):
            # ---------------- constants to SBUF ----------------
            def load_const(ap, shape):
                t = cpool.tile(list(shape), F32, tag=ap.name)
                nc.sync.dma_start(t[:], ap[:])
                return t

            enc_W1_s = load_const(enc_W1, (10, HID))
            enc_b1_s = load_const(enc_b1c, (HID, 1))
            enc_W2_s = load_const(enc_W2, (HID, HID))
            enc_b2_s = load_const(enc_b2c, (HID, 1))
            ones_s = load_const(ones128, (1, 128))
            W_s, a1_s, a2_s = {}, {}, {}
            for layer in range(3):
                kcn = max(1, FIN[layer] // 128)
                for h in range(H):
                    if FIN[layer] <= 128:
                        W_s[layer, h] = load_const(
                            gat_W[layer, h], (FIN[layer], FO[layer]))
                    else:
                        t = cpool.tile([128, kcn, FO[layer]], F32,
                                       tag=f"W_{layer}_{h}")
                        nc.sync.dma_start(
                            t[:],
                            gat_W[layer, h].rearrange("(kc kp) f -> kp kc f",
                                                      kp=128))
                        W_s[layer, h] = t
                    a1_s[layer, h] = load_const(gat_a1r[layer, h],
                                                (128, FO[layer]))
                    a2_s[layer, h] = load_const(gat_a2[layer, h],
                                                (FO[layer], 1))
            W1blk_s = load_const(W1blk, (128, 128))
            W2blk_s = load_const(W2blk, (128, 32))
            b1c_s = load_const(b1c, (128, 1))
            b2c_s = load_const(b2c, (128, 1))

            # adjacency mask, transposed: [q in chunk, chunk, p_local]
            adjTf = big.tile([128, 16, NL], F32, tag="adjTf")
            nc.sync.dma_start(
                adjTf[:], adjT_loc.rearrange("(qc qp) p -> qp qc p", qc=16))

            # ---------------- encoder (local rows) ----------------
            nfT_s = work.tile([10, NL], F32, tag="nfT")
            nc.sync.dma_start(nfT_s[:], nfT_loc[:])
            ps0 = ps_mm.tile([128, 512], F32, tag="mm")
            nc.tensor.matmul(ps0[:HID, :NL], enc_W1_s[:], nfT_s[:],
                             start=True, stop=True)
            x0 = work.tile([HID, NL], F32, tag="x0")
            nc.scalar.activation(x0[:], ps0[:HID, :NL], AF.Relu,
                                 bias=enc_b1_s[:, 0:1])
            ps1 = ps_mm.tile([128, 512], F32, tag="mm")
            nc.tensor.matmul(ps1[:HID, :NL], enc_W2_s[:], x0[:],
                             start=True, stop=True)
            xloc = work.tile([HID, NL], F32, tag="xloc0")
            nc.vector.tensor_scalar_add(xloc[:], ps1[:HID, :NL],
                                        enc_b2_s[:, 0:1])

            # ---------------- encoder all-gather ----------------
            agin0 = dram.tile([HID, NL], F32, tag="agin_x0")
            agout0 = dram.tile([NC, HID, NL], F32, tag="agout_x0",
                               addr_space="Shared")
            nc.sync.dma_start(agin0[:], xloc[:])
            nc.gpsimd.collective_compute(
                "AllGather", ALU.bypass, replica_groups=RG,
                ins=[agin0[:]], outs=[agout0[:]])
            xt0 = big.tile([HID, N], F32, tag="xT0")
            nc.sync.dma_start(xt0.rearrange("f (r p) -> f r p", r=NC),
                              agout0.rearrange("r f p -> f r p"))
            xT_chunks = [xt0]
            xloc_chunks = [xloc]

            def elu_chain(dst, src_ap, shape, tag):
                r = work.tile(list(shape), F32, tag=f"elu_r")
                nc.scalar.activation(r[:], src_ap, AF.Relu)
                d = work.tile(list(shape), F32, tag=f"elu_d")
                nc.vector.tensor_sub(d[:], src_ap, r[:])
                nc.scalar.activation(d[:], d[:], AF.Exp)
                nc.vector.tensor_add(dst, r[:], d[:])
                nc.vector.tensor_scalar_add(dst, dst, -1.0)

            emb_agout = None
            embT_loc_s = None

            # ---------------- GAT layers ----------------
            for layer in range(3):
                fin, fo = FIN[layer], FO[layer]
                kcn = len(xT_chunks)
                concat = layer < 2
                if concat:
                    xnT = work.tile([128, 2, NL], F32, tag=f"xnT_{layer}")
                else:
                    embacc = work.tile([OUT, NL], F32, tag="embacc")

                for h in range(H):
                    Wt = W_s[layer, h]

                    def w_chunk(kc):
                        return Wt[:, kc, :] if kcn > 1 else Wt[:]

                    # h_aug [128, 16, ones_col+1]: normal-layout h + ones
                    # column; ones_col is 32-aligned so the PSUM row of the
                    # softmax denominator is engine-readable.
                    ones_col = fo if fo % 32 == 0 else ((fo // 32) + 1) * 32
                    augw = ones_col + 1
                    haug = work.tile([128, 16, augw], F32, tag="haug")
                    if ones_col > fo:
                        nc.vector.memset(haug[:, :, fo:ones_col], 0.0)
                    nc.vector.memset(haug[:, :, ones_col:augw], 1.0)
                    for qc in range(16):
                        psh = ps_mm.tile([128, 512], F32, tag="mm")
                        for kc in range(kcn):
                            nc.tensor.matmul(
                                psh[:, :fo],
                                xT_chunks[kc][:, qc * 128:(qc + 1) * 128],
                                w_chunk(kc),
                                start=(kc == 0), stop=(kc == kcn - 1))
                        nc.vector.tensor_copy(haug[:, qc, :fo], psh[:, :fo])

                    # s1P [128, 16] = h . a1   (partition = q layout)
                    s1tmp = work.tile([128, 16, fo], F32, tag="s1tmp")
                    nc.vector.tensor_mul(
                        s1tmp[:], haug[:, :, :fo],
                        a1_s[layer, h][:, None, :].to_broadcast((128, 16, fo)))
                    s1P = work.tile([128, 16], F32, tag="s1P")
                    nc.vector.reduce_sum(s1P[:], s1tmp[:],
                                         axis=mybir.AxisListType.X)

                    # s2 for local rows from local activations
                    pshl = ps_mm.tile([128, 512], F32, tag="mm")
                    for kc in range(kcn):
                        nc.tensor.matmul(pshl[:fo, :NL], w_chunk(kc),
                                         xloc_chunks[kc][:],
                                         start=(kc == 0), stop=(kc == kcn - 1))
                    hT_loc = work.tile([fo, NL], F32, tag="hTloc")
                    nc.vector.tensor_copy(hT_loc[:], pshl[:fo, :NL])
                    pss2 = ps_mm.tile([128, 512], F32, tag="mm")
                    nc.tensor.matmul(pss2[:1, :NL], a2_s[layer, h][:],
                                     hT_loc[:], start=True, stop=True)
                    s2row = work.tile([1, NL], F32, tag="s2row")
                    nc.vector.tensor_copy(s2row[:], pss2[:1, :NL])
                    pss2r = ps_s2.tile([128, NL], F32, tag="s2rep")
                    nc.tensor.matmul(pss2r[:], ones_s[:], s2row[:],
                                     start=True, stop=True)

                    # attention, transposed: pexpm[q, p] =
                    #   exp(lrelu(s1[q] + s2[p])) * adjT[q, p]
                    pvps = ps_pv.tile([augw, NL], F32, tag="pv")
                    for qc in range(16):
                        lr = attp.tile([128, NL], F32, tag="lr")
                        nc.scalar.activation(lr[:], pss2r[:], AF.Prelu,
                                             bias=s1P[:, qc:qc + 1],
                                             alpha=ALPHA)
                        pex = attp.tile([128, NL], F32, tag="pex")
                        nc.scalar.activation(pex[:], lr[:], AF.Exp)
                        pexm = attp.tile([128, NL], F32, tag="pexm")
                        nc.vector.tensor_mul(pexm[:], pex[:], adjTf[:, qc, :])
                        nc.tensor.matmul(pvps[:], haug[:, qc, :], pexm[:],
                                         start=(qc == 0), stop=(qc == 15))

                    # normalize by S (ones row of the PV matmul)
                    ssb = work.tile([1, NL], F32, tag="ssb")
                    nc.vector.tensor_copy(ssb[:], pvps[ones_col:ones_col + 1, :])
                    psrep = ps_mm.tile([128, 512], F32, tag="mm")
                    nc.tensor.matmul(psrep[:fo, :NL], ones_s[:, :fo], ssb[:],
                                     start=True, stop=True)
                    rec = work.tile([fo, NL], F32, tag="rec")
                    nc.vector.reciprocal(rec[:], psrep[:fo, :NL])
                    if concat:
                        dst = xnT[(h % 2) * 64:(h % 2) * 64 + 64, h // 2, :]
                        nc.vector.tensor_mul(dst, pvps[:fo, :], rec[:])
                    else:
                        hp = work.tile([OUT, NL], F32, tag="hp2")
                        nc.vector.tensor_mul(hp[:], pvps[:fo, :], rec[:])
                        if h == 0:
                            nc.vector.tensor_copy(embacc[:], hp[:])
                        else:
                            nc.vector.tensor_add(embacc[:], embacc[:], hp[:])

                if concat:
                    xn_elu = work.tile([128, 2, NL], F32, tag=f"xnelu_{layer}")
                    elu_chain(xn_elu[:], xnT[:], (128, 2, NL), f"elu{layer}")
                    agin = dram.tile([2, 128, NL], F32, tag=f"agin_x{layer + 1}")
                    agout = dram.tile([NC, 2, 128, NL], F32,
                                      tag=f"agout_x{layer + 1}",
                                      addr_space="Shared")
                    for c in range(2):
                        nc.sync.dma_start(agin[c, :, :], xn_elu[:, c, :])
                    nc.gpsimd.collective_compute(
                        "AllGather", ALU.bypass, replica_groups=RG,
                        ins=[agin[:]], outs=[agout[:]])
                    xT_chunks = []
                    for kc in range(2):
                        t = big.tile([128, N], F32, tag=f"xTa_{kc}")
                        nc.sync.dma_start(
                            t.rearrange("f (r p) -> f r p", r=NC),
                            agout[:, kc, :, :].rearrange("r f p -> f r p"))
                        xT_chunks.append(t)
                    xloc_chunks = [xn_elu[:, 0, :], xn_elu[:, 1, :]]
                else:
                    nc.vector.tensor_scalar_mul(embacc[:], embacc[:], 1.0 / H)
                    embT_loc_s = work.tile([OUT, NL], F32, tag="embT_l")
                    elu_chain(embT_loc_s[:], embacc[:], (OUT, NL), "elu_emb")
                    nc.sync.dma_start(embT_out[:], embT_loc_s[:])
                    agin_e = dram.tile([OUT, NL], F32, tag="agin_emb")
                    emb_agout = dram.tile([NC, OUT, NL], F32, tag="agout_emb",
                                          addr_space="Shared")
                    nc.sync.dma_start(agin_e[:], embT_loc_s[:])
                    nc.gpsimd.collective_compute(
                        "AllGather", ALU.bypass, replica_groups=RG,
                        ins=[agin_e[:]], outs=[emb_agout[:]])

            # ---------------- edge classifier ----------------
            # 32-stride packing (PE quadrant rule): partition 32 j + 3 g + c,
            # j = p within group-of-4, rows 32 j + 12 .. 32 j + 31 are zero.
            # emb_i [128, 512]: -> embT[c, 512 g + qg]
            # from emb_agout [8, 3, 256]; rank axis = (g, q1) pairs.
            emb_i = big.tile([128, 512], F32, tag="emb_i")
            nc.vector.memset(emb_i[:], 0.0)
            for j in range(4):
                for g in range(4):
                    for q1 in range(2):
                        nc.sync.dma_start(
                            emb_i[32 * j + 3 * g:32 * j + 3 * g + 3,
                                  256 * q1:256 * q1 + 256],
                            emb_agout[2 * g + q1])
            # embP [128, 64]: col u -> -emb[4 u + j, c] at partition (j, g, c)
            nege = work.tile([OUT, NL], F32, tag="nege")
            nc.vector.tensor_scalar_mul(nege[:], embT_loc_s[:], -1.0)
            negd = dram.tile([OUT, NL], F32, tag="negd")
            nc.sync.dma_start(negd[:], nege[:])
            embP = big.tile([128, 64], F32, tag="embP")
            nc.vector.memset(embP[:], 0.0)
            psrc = negd.rearrange("c (u j) -> j c u", j=4)
            for j in range(4):
                for g in range(4):
                    nc.sync.dma_start(
                        embP[32 * j + 3 * g:32 * j + 3 * g + 3, :],
                        psrc[j])

            # ep rows p = 4 u + j ; cols q = 512 g' + qg.
            # stage-2 writes [32, 512] (28 zero rows via zero cols of W2blk)
            # at psum partition 32 j; each probs bank holds 4 p rows.
            if kdebug:
                nc.sync.dma_start(dbg_emb_i[:], emb_i[:])
                nc.sync.dma_start(dbg_embP[:], embP[:])
            for u in range(64):
                adf = clsp.tile([128, 512], F32, tag="adf")
                nc.scalar.activation(adf[:], emb_i[:], AF.Abs,
                                     bias=embP[:, u:u + 1])
                if kdebug and u == 0:
                    nc.sync.dma_start(dbg_adf[:], adf[:])
                psp = ps_pr.tile([128, 512], F32, tag="probs")
                for j in range(4):
                    psh = ps_mm.tile([128, 512], F32, tag="mm")
                    nc.tensor.matmul(psh[:], W1blk_s[32 * j:32 * j + 12, :],
                                     adf[32 * j:32 * j + 12, :],
                                     start=True, stop=True,
                                     tile_position=(32 * j, 0))
                    hid = clsp.tile([128, 512], F32, tag="hid")
                    nc.vector.tensor_scalar(hid[:], psh[:],
                                            b1c_s[:, 0:1], 0.0,
                                            ALU.add, ALU.max)
                    if kdebug and u == 0 and j == 0:
                        nc.sync.dma_start(dbg_hid[:], hid[:])
                    nc.tensor.matmul(psp[32 * j:32 * j + 32, :], W2blk_s[:],
                                     hid[:], start=True, stop=True,
                                     tile_position=(0, 32 * j))
                probs = clsp.tile([128, 512], F32, tag="probs_sb")
                nc.scalar.activation(probs[:], psp[:], AF.Sigmoid,
                                     bias=b2c_s[:, 0:1])
                if kdebug and u == 0:
                    nc.sync.dma_start(dbg_probs[:], probs[:])
                # real rows: partition 32 j + g' -> ep[4 u + j, 512 g' + qg]
                for j in range(4):
                    nc.sync.dma_start(
                        ep_out[4 * u + j, :].rearrange("(g q) -> g q", g=4),
                        probs[32 * j:32 * j + 4, :])

    nc.compile()
    return nc


def _prep_inputs(node_features, adj_matrix, enc_W1, enc_b1, enc_W2, enc_b2,
                 gat_W0, gat_a0, gat_W1, gat_a1, gat_W2, gat_a2,
                 cls_W1, cls_b1, cls_W2, cls_b2):
    nf = np.asarray(node_features, np.float32)
    adj = np.asarray(adj_matrix)
    gw = [np.asarray(gat_W0, np.float32), np.asarray(gat_W1, np.float32),
          np.asarray(gat_W2, np.float32)]
    ga = [np.asarray(gat_a0, np.float32), np.asarray(gat_a1, np.float32),
          np.asarray(gat_a2, np.float32)]

    common = {
        "enc_W1": np.ascontiguousarray(np.asarray(enc_W1, np.float32)),
        "enc_b1c": np.ascontiguousarray(
            np.asarray(enc_b1, np.float32)[:, None]),
        "enc_W2": np.ascontiguousarray(np.asarray(enc_W2, np.float32)),
        "enc_b2c": np.ascontiguousarray(
            np.asarray(enc_b2, np.float32)[:, None]),
        "ones128": np.ones((1, 128), np.float32),
    }
    for layer in range(3):
        fo = FO[layer]
        for h in range(H):
            a1 = ga[layer][h, :fo, 0]
            a2 = ga[layer][h, fo:, 0]
            common[f"gat_W_{layer}_{h}"] = np.ascontiguousarray(gw[layer][h])
            common[f"gat_a1r_{layer}_{h}"] = np.ascontiguousarray(
                np.broadcast_to(a1[None, :], (128, fo)).astype(np.float32))
            common[f"gat_a2_{layer}_{h}"] = np.ascontiguousarray(
                a2[:, None].astype(np.float32))

    w1 = np.asarray(cls_W1, np.float32)          # (3, 32)
    b1 = np.asarray(cls_b1, np.float32)          # (32,)
    w2 = np.asarray(cls_W2, np.float32)          # (32, 1)
    b2 = np.asarray(cls_b2, np.float32)          # (1,)
    W1blk = np.zeros((128, 128), np.float32)
    W2blk = np.zeros((128, 32), np.float32)
    b1c = np.zeros((128, 1), np.float32)
    for g in range(4):
        for j in range(4):
            W1blk[32 * j + 3 * g:32 * j + 3 * g + 3, 32 * g:32 * g + 32] = w1
        W2blk[32 * g:32 * g + 32, g] = w2[:, 0]
        b1c[32 * g:32 * g + 32, 0] = b1
    common["W1blk"] = W1blk
    common["W2blk"] = W2blk
    common["b1c"] = b1c
    common["b2c"] = np.full((128, 1), float(b2[0]), np.float32)

    maps = []
    for r in range(NC):
        r0 = r * NL
        m = dict(common)
        m["nfT_loc"] = np.ascontiguousarray(nf[r0:r0 + NL].T)
        m["adjT_loc"] = np.ascontiguousarray(
            (adj[r0:r0 + NL] > 0).astype(np.float32).T)
        maps.append(m)
    return maps


LAST_RESULT = None


def kernel(**inputs):
    global LAST_RESULT
    maps = _prep_inputs(**inputs)
    if "nc" not in _CACHE:
        _CACHE["nc"] = _build()
    res = bass_utils.run_bass_kernel_spmd(
        _CACHE["nc"], maps, core_ids=list(range(NC)),
        trace=os.environ.get("BASS_TRACE", "0") not in ("", "0"))
    LAST_RESULT = res
    emb = np.concatenate(
        [res.results[r]["embT_loc"].T for r in range(NC)], axis=0)
    ep = np.concatenate(
        [res.results[r]["ep_loc"] for r in range(NC)], axis=0)
    return np.ascontiguousarray(emb), np.ascontiguousarray(ep)
